# revision 11
# baseline (speedup 1.0000x reference)
"""Cosine multihead attention on 8 Trainium2 NeuronCores.

Sharding: batch*heads across cores. Core c handles batch b = c // 4 and the
4 heads [4*(c%4), 4*(c%4)+4). Each core computes its heads' q/k/v projections
(tensor-parallel slices of in_proj), full attention for its (B,H) slice, and a
partial out-projection (rank-256 contribution, bf16). The host sums the 4
partials per batch in fp32 and adds out_proj_bias.

Key structure (v2):
- q projected transposed [dims, seq], L2-normalized on-chip (bf16).
- k projected transposed but NOT normalized: 1/(tau*||k||) is folded into the
  softmax exp as the ACT per-partition scale operand (partitions = keys), via
  sqrt(ss*tau^-2) -> bf16 -> DMA-transpose -> DVE reciprocal.
- v projected transposed then moved to natural [keys, head, 65] layout with 4
  big DMA-transposes; column 64 holds ones so PV also accumulates softmax
  denominators (M=65).
- Attention in 1024-query superblocks: per (pair, half, kc) two QK matmuls on
  concurrent 64-row PE tiles (1024-wide moving), exp per head on [128,1024],
  then two PV matmuls (1024-wide moving).
- A subset of exp tiles is computed on the Vector engine with a Schraudolph
  bf16 bit-trick (t = s*C/(tau*||k||) + B; round to int16; bits are bf16) to
  take load off the Scalar engine, which is otherwise the bottleneck.
- Out-projection in 1024-wide units, bf16 partial written to HBM.
"""

import sys

if "/opt/trn_rl_repo" not in sys.path:
    sys.path.insert(0, "/opt/trn_rl_repo")

import numpy as np
import ml_dtypes

import concourse.bass as bass
import concourse.tile as tile
from concourse import bacc, mybir
from concourse.bass_utils import run_bass_kernel_spmd

S, B, E, H = 2048, 2, 1024, 16
HD = E // H            # 64
HPC = 4                # heads per core
NCORES = 8
TAU_MIN = 0.01

BF16 = ml_dtypes.bfloat16
DT_BF = mybir.dt.bfloat16
DT_F32 = mybir.dt.float32
DT_I16 = mybir.dt.int16

KC_E = E // 128        # 8 contraction chunks for projections
MQ = S // 128          # 16 seq chunks of 128
NPAIR = HPC // 2       # 2 head pairs per core
NKC = S // 128         # 16 key chunks in attention

# Schraudolph exp-on-DVE: which kc chunks of each (pair, half) round go to the
# vector engine instead of ACT.  4/16 keeps the extra error ~2.4e-3 while
# cutting ACT exp work by 25%.
SCHR_KC = frozenset((3, 7, 11, 14))
SCHR_C = 128.0 * 1.4426950408889634   # 2^7 * log2(e)
SCHR_B = 127.0 * 128.0 - 6.0          # exponent bias - error-balancing shift


def build_program():
    nc = bacc.Bacc(None)

    xq = nc.dram_tensor("xq_t", [E, S], DT_BF, kind="ExternalInput")
    xk = nc.dram_tensor("xk_t", [E, S], DT_BF, kind="ExternalInput")
    xv = nc.dram_tensor("xv_t", [E, S], DT_BF, kind="ExternalInput")
    wq = nc.dram_tensor("wq_t", [E, 256], DT_BF, kind="ExternalInput")
    wk = nc.dram_tensor("wk_t", [E, 256], DT_BF, kind="ExternalInput")
    wv = nc.dram_tensor("wv_t", [E, 256], DT_BF, kind="ExternalInput")
    bq = nc.dram_tensor("b_q", [1, 256], DT_BF, kind="ExternalInput")
    bk = nc.dram_tensor("b_k", [1, 256], DT_BF, kind="ExternalInput")
    bv = nc.dram_tensor("b_v", [1, 256], DT_BF, kind="ExternalInput")
    wo = nc.dram_tensor("wo_t", [256, E], DT_BF, kind="ExternalInput")
    selq_in = nc.dram_tensor("selq", [2, 128], DT_F32, kind="ExternalInput")
    tau2_in = nc.dram_tensor("tau2i", [2, 2], DT_F32, kind="ExternalInput")
    outp = nc.dram_tensor("out_p", [S, E], DT_BF, kind="ExternalOutput")
    DEBUG = bool(__import__("os").environ.get("KDEBUG"))
    if DEBUG:
        dbg_qt = nc.dram_tensor("dbg_qt", [2, 128, S], DT_BF, kind="ExternalOutput")
        dbg_kt = nc.dram_tensor("dbg_kt", [2, 128, S], DT_BF, kind="ExternalOutput")
        dbg_rkt = nc.dram_tensor("dbg_rkt", [128, NPAIR, 16, 2], DT_F32, kind="ExternalOutput")
        dbg_v = nc.dram_tensor("dbg_v", [128, 16, 4, 65], DT_BF, kind="ExternalOutput")
        dbg_ht = nc.dram_tensor("dbg_ht", [2, 128, S], DT_BF, kind="ExternalOutput")

    with tile.TileContext(nc) as tc:
        with (
            tc.tile_pool(name="consts", bufs=1) as consts,
            tc.tile_pool(name="xin", bufs=1) as xin,
            tc.tile_pool(name="xvp", bufs=1) as xvp,
            tc.tile_pool(name="wts", bufs=1) as wts,
            tc.tile_pool(name="qk", bufs=1) as qkpool,
            tc.tile_pool(name="norm", bufs=1) as normpool,
            tc.tile_pool(name="work", bufs=2) as work,
            tc.tile_pool(name="sqp", bufs=2) as sqp,
            tc.tile_pool(name="expool", bufs=4) as expool,
            tc.tile_pool(name="zwork", bufs=1) as zwork,
            tc.tile_pool(name="outs", bufs=2) as outs,
            tc.tile_pool(name="ps_a", bufs=2, space="PSUM") as ps_a,
            tc.tile_pool(name="ps_b", bufs=2, space="PSUM") as ps_b,
        ):
            # ---- constants -------------------------------------------------
            ones_row = consts.tile([1, 1024], DT_BF, tag="ones_row")
            nc.vector.memset(ones_row, 1.0)
            selq = consts.tile([2, 128], DT_F32, tag="selq")
            nc.sync.dma_start(out=selq, in_=selq_in[:, :])
            tau2_sb = consts.tile([2, 2], DT_F32, tag="tau2")
            nc.sync.dma_start(out=tau2_sb, in_=tau2_in[:, :])
            hsel = consts.tile([128, 2], DT_BF, tag="hsel")
            nc.vector.memset(hsel, 0.0)
            nc.vector.memset(hsel[0:64, 0:1], 1.0)
            nc.vector.memset(hsel[64:128, 1:2], 1.0)
            ones_hi = consts.tile([128, 64], DT_F32, tag="ones_hi")
            nc.vector.memset(ones_hi, 1.0)

            # ---- weights ---------------------------------------------------
            wq_sb = wts.tile([128, KC_E, 256], DT_BF, tag="wq")
            wk_sb = wts.tile([128, KC_E, 256], DT_BF, tag="wk")
            wv_sb = wts.tile([128, KC_E, 256], DT_BF, tag="wv")
            bq_sb = consts.tile([1, 256], DT_BF, tag="bq")
            bk_sb = consts.tile([1, 256], DT_BF, tag="bk")
            bv_sb = consts.tile([1, 256], DT_BF, tag="bv")
            nc.sync.dma_start(out=bq_sb, in_=bq[:, :])
            nc.sync.dma_start(out=bk_sb, in_=bk[:, :])
            nc.sync.dma_start(out=bv_sb, in_=bv[:, :])
            for c in range(KC_E):
                nc.sync.dma_start(out=wq_sb[:, c, :], in_=wq[c * 128:(c + 1) * 128, :])
                nc.gpsimd.dma_start(out=wk_sb[:, c, :], in_=wk[c * 128:(c + 1) * 128, :])
                nc.gpsimd.dma_start(out=wv_sb[:, c, :], in_=wv[c * 128:(c + 1) * 128, :])
            wo_sb = wts.tile([128, 2, E], DT_BF, tag="wo")
            for c in range(2):
                nc.gpsimd.dma_start(out=wo_sb[:, c, :], in_=wo[c * 128:(c + 1) * 128, :])

            # ---- activations: xq first (q-proj starts earliest) ------------
            xq_sb = xin.tile([128, KC_E, S], DT_BF, tag="xq")
            xk_sb = xin.tile([128, KC_E, S], DT_BF, tag="xk")
            for c in range(KC_E):
                nc.sync.dma_start(out=xq_sb[:, c, :], in_=xq[c * 128:(c + 1) * 128, :])
                nc.gpsimd.dma_start(out=xk_sb[:, c, :], in_=xk[c * 128:(c + 1) * 128, :])
            # xv shares its pool slot with v_sb later (xv is dead once the
            # transposed v projection has consumed it).
            xv_sb = xvp.tile([128, KC_E, S], DT_BF, tag="xv", name="xv_sb")
            for c in range(KC_E):
                nc.sync.dma_start(out=xv_sb[:, c, :], in_=xv[c * 128:(c + 1) * 128, :])

            qt = [qkpool.tile([128, S], DT_BF, tag=f"qt{p}", name=f"qt{p}")
                  for p in range(NPAIR)]
            kt = [qkpool.tile([128, S], DT_BF, tag=f"kt{p}", name=f"kt{p}")
                  for p in range(NPAIR)]
            heads_t = [qkpool.tile([128, S], DT_BF, tag=f"ht{p}", name=f"ht{p}")
                       for p in range(NPAIR)]
            vT_sb = qkpool.tile([128, 2, S], DT_BF, tag="vT")

            # k norms (padded to 16 partitions for the transpose DMA) and the
            # transposed reciprocal tiles used as exp scales.
            krn_sb = normpool.tile([16, NPAIR, S], DT_BF, tag="krn")
            nc.gpsimd.memset(krn_sb, 1.0)
            krt = normpool.tile([128, NPAIR, NKC, 16], DT_BF, tag="krt")
            rkt = normpool.tile([128, NPAIR, NKC, 2], DT_F32, tag="rkt")
            rkc = normpool.tile([128, NPAIR, NKC, 2], DT_F32, tag="rkc")

            def proj_mms(pp, w_sb, b_sb, x_sb, mcol, half):
                # matmul out must fit one PSUM bank: split free dim in two,
                # sharing the stationary (one LDWEIGHTS per c chunk).
                for c in range(KC_E):
                    for hh in range(2):
                        nc.tensor.matmul(
                            pp[:, hh * 512:(hh + 1) * 512],
                            lhsT=w_sb[:, c, mcol * 128:(mcol + 1) * 128],
                            rhs=x_sb[:, c,
                                     half * 1024 + hh * 512:
                                     half * 1024 + (hh + 1) * 512],
                            start=(c == 0),
                            stop=False,
                        )
                for hh in range(2):
                    nc.tensor.matmul(
                        pp[:, hh * 512:(hh + 1) * 512],
                        lhsT=b_sb[0:1, mcol * 128:(mcol + 1) * 128],
                        rhs=ones_row[0:1, hh * 512:(hh + 1) * 512],
                        start=False,
                        stop=True,
                    )

            def q_unit(mc, half):
                sl = slice(half * 1024, (half + 1) * 1024)
                pp = ps_a.tile([128, 1024], DT_F32, tag="a", name="pp_q")
                proj_mms(pp, wq_sb, bq_sb, xq_sb, mc, half)
                nc.vector.tensor_copy(out=qt[mc][:, sl], in_=pp)
                sq = sqp.tile([128, 1024], DT_BF, tag="sq", name="sq_q")
                nc.vector.tensor_mul(sq, qt[mc][:, sl], qt[mc][:, sl])
                ss = ps_b.tile([2, 1024], DT_F32, tag="b", name="ss_q")
                for hh in range(2):
                    nc.tensor.matmul(
                        ss[:, hh * 512:(hh + 1) * 512], lhsT=hsel,
                        rhs=sq[:, hh * 512:(hh + 1) * 512],
                        start=True, stop=True)
                st = work.tile([2, 1024], DT_F32, tag="st", name="st_q")
                nc.scalar.activation(st, ss, mybir.ActivationFunctionType.Sqrt)
                rb = ps_b.tile([128, 1024], DT_F32, tag="b", name="rb_q")
                for hh in range(2):
                    nc.tensor.matmul(
                        rb[:, hh * 512:(hh + 1) * 512], lhsT=selq,
                        rhs=st[:, hh * 512:(hh + 1) * 512],
                        start=True, stop=True)
                rq = work.tile([128, 1024], DT_F32, tag="rq", name="rq_q")
                nc.vector.reciprocal_approx_fast(out=rq, in_=rb)
                nc.vector.tensor_mul(qt[mc][:, sl], qt[mc][:, sl], rq)

            def k_unit(mc, half):
                sl = slice(half * 1024, (half + 1) * 1024)
                pp = ps_a.tile([128, 1024], DT_F32, tag="a", name="pp_k")
                proj_mms(pp, wk_sb, bk_sb, xk_sb, mc, half)
                nc.vector.tensor_copy(out=kt[mc][:, sl], in_=pp)
                sq = sqp.tile([128, 1024], DT_BF, tag="sq", name="sq_k")
                nc.vector.tensor_mul(sq, kt[mc][:, sl], kt[mc][:, sl])
                ss = ps_b.tile([2, 1024], DT_F32, tag="b", name="ss_k")
                for hh in range(2):
                    nc.tensor.matmul(
                        ss[:, hh * 512:(hh + 1) * 512], lhsT=hsel,
                        rhs=sq[:, hh * 512:(hh + 1) * 512],
                        start=True, stop=True)
                # krn = sqrt(ss * tau^2) = tau * ||k||   (bf16)
                nc.scalar.activation(
                    krn_sb[0:2, mc, sl], ss,
                    mybir.ActivationFunctionType.Sqrt,
                    scale=tau2_sb[:, mc:mc + 1],
                )

            def v_unit(d, half):
                sl = slice(half * 1024, (half + 1) * 1024)
                pp = ps_a.tile([128, 1024], DT_F32, tag="a", name="pp_v")
                proj_mms(pp, wv_sb, bv_sb, xv_sb, d, half)
                nc.vector.tensor_copy(out=vT_sb[:, d, sl], in_=pp)

            # ---- all projections up front ----------------------------------
            for half in range(2):
                q_unit(0, half)
            for half in range(2):
                k_unit(0, half)
            for half in range(2):
                q_unit(1, half)
            for half in range(2):
                k_unit(1, half)
            for d in range(2):
                for half in range(2):
                    v_unit(d, half)

            # k-norm reciprocals: transpose then reciprocal (fp32)
            for p in range(NPAIR):
                nc.sync.dma_start_transpose(out=krt[:, p], in_=krn_sb[:, p, :])
                krf = work.tile([128, NKC, 2], DT_F32, tag="st", name="krf")
                nc.vector.tensor_copy(out=krf, in_=krt[:, p, :, 0:2])
                nc.vector.reciprocal_approx_fast(out=rkt[:, p], in_=krf)
                nc.vector.tensor_scalar_mul(rkc[:, p], rkt[:, p], SCHR_C)

            # v into natural [keys, kc, head, 65] layout; col 64 = ones.
            # DMA transpose needs a contiguous destination, so land in v2
            # first and strided-copy into place on the vector engine.
            v_sb = xvp.tile([128, NKC, HPC, HD + 1], DT_BF, tag="xv", name="v_sb")
            nc.gpsimd.memset(v_sb[:, :, :, HD:HD + 1], 1.0)
            for d in range(2):
                v2 = work.tile([128, NKC, 128], DT_BF, tag="rq", name="v2")
                nc.scalar.dma_start_transpose(out=v2, in_=vT_sb[:, d, :])
                for j in range(2):
                    nc.vector.tensor_copy(
                        out=v_sb[:, :, 2 * d + j, 0:HD],
                        in_=v2[:, :, j * 64:(j + 1) * 64],
                    )

            # ---- attention -------------------------------------------------
            def attention_round(p, half):
                sl_q = slice(half * 1024, (half + 1) * 1024)
                o = [ps_b.tile([65, 1024], DT_F32, tag="b", name=f"o{j}")
                     for j in range(2)]
                for kc in range(NKC):
                    sc = [ps_a.tile([128, 1024], DT_F32, tag="a", name=f"sc{j}")
                          for j in range(2)]
                    for j in range(2):
                        rows = slice(j * 64, (j + 1) * 64)
                        for hh in range(2):
                            nc.tensor.matmul(
                                sc[j][:, hh * 512:(hh + 1) * 512],
                                lhsT=kt[p][rows, kc * 128:(kc + 1) * 128],
                                rhs=qt[p][rows,
                                          half * 1024 + hh * 512:
                                          half * 1024 + (hh + 1) * 512],
                                start=True, stop=True,
                            )
                    exs = []
                    for j in range(2):
                        ex = expool.tile([128, 1024], DT_BF, tag="ex", name=f"ex{j}")
                        if kc in SCHR_KC:
                            tf = work.tile([128, 1024], DT_F32, tag="rq", name="tf")
                            nc.vector.tensor_scalar(
                                out=tf, in0=sc[j],
                                scalar1=rkc[:, p, kc, j:j + 1],
                                scalar2=SCHR_B,
                                op0=mybir.AluOpType.mult,
                                op1=mybir.AluOpType.add,
                            )
                            nc.vector.tensor_copy(
                                out=ex.bitcast(DT_I16), in_=tf
                            )
                        else:
                            nc.scalar.activation(
                                ex, sc[j],
                                mybir.ActivationFunctionType.Exp,
                                scale=rkt[:, p, kc, j:j + 1],
                            )
                        exs.append(ex)
                    for j in range(2):
                        for hh in range(2):
                            nc.tensor.matmul(
                                o[j][0:65, hh * 512:(hh + 1) * 512],
                                lhsT=v_sb[:, kc, 2 * p + j, :],
                                rhs=exs[j][:, hh * 512:(hh + 1) * 512],
                                start=(kc == 0), stop=(kc == NKC - 1),
                            )
                for j in range(2):
                    rows = slice(j * 64, (j + 1) * 64)
                    # z lives in PSUM partition 64; engines cannot shift
                    # partitions, so copy it to SBUF and PE-broadcast it
                    # down to partitions 0-63.
                    zs = zwork.tile([128, 1024], DT_F32, tag="rz", name="zs")
                    nc.vector.tensor_copy(zs[64:65, :], o[j][64:65, :])
                    zb = ps_a.tile([64, 1024], DT_F32, tag="a", name="zb")
                    for hh in range(2):
                        nc.tensor.matmul(
                            zb[:, hh * 512:(hh + 1) * 512],
                            lhsT=ones_hi[64:65, 0:64],
                            rhs=zs[64:65, hh * 512:(hh + 1) * 512],
                            start=True, stop=True,
                        )
                    zbi = zwork.tile([64, 1024], DT_F32, tag="zb", name="zbi")
                    nc.vector.reciprocal_approx_fast(out=zbi, in_=zb)
                    nc.vector.tensor_mul(
                        heads_t[p][rows, sl_q], o[j][0:64, :], zbi
                    )

            def outproj_unit(m):
                op = ps_a.tile([128, 1024], DT_F32, tag="a", name="op")
                for c in range(2):
                    for hh in range(2):
                        nc.tensor.matmul(
                            op[:, hh * 512:(hh + 1) * 512],
                            lhsT=heads_t[c][:, m * 128:(m + 1) * 128],
                            rhs=wo_sb[:, c, hh * 512:(hh + 1) * 512],
                            start=(c == 0), stop=(c == 1),
                        )
                ob = outs.tile([128, 1024], DT_BF, tag="ob", name="ob")
                nc.vector.tensor_copy(ob, op)
                nc.sync.dma_start(out=outp[m * 128:(m + 1) * 128, :], in_=ob)

            if DEBUG:
                for p in range(NPAIR):
                    nc.sync.dma_start(out=dbg_qt[p], in_=qt[p][:, :])
                    nc.sync.dma_start(out=dbg_kt[p], in_=kt[p][:, :])
                nc.sync.dma_start(out=dbg_rkt[:, :, :, :], in_=rkt[:, :, :, :])
                nc.sync.dma_start(out=dbg_v[:, :, :, :], in_=v_sb[:, :, :, :])
            attention_round(0, 0)
            attention_round(0, 1)
            attention_round(1, 0)
            for m in range(8):
                outproj_unit(m)
            attention_round(1, 1)
            for m in range(8, 16):
                outproj_unit(m)
            if DEBUG:
                for p in range(NPAIR):
                    nc.sync.dma_start(out=dbg_ht[p], in_=heads_t[p][:, :])

    nc.compile()
    return nc


_CACHE = {}


def _get_program():
    if "nc" not in _CACHE:
        _CACHE["nc"] = build_program()
    return _CACHE["nc"]


def make_in_maps(query, key, value, in_proj_weight, in_proj_bias,
                 out_proj_weight, out_proj_bias, tau):
    query = np.asarray(query, np.float32)
    key = np.asarray(key, np.float32)
    value = np.asarray(value, np.float32)
    W = np.asarray(in_proj_weight, np.float32)
    bias = np.asarray(in_proj_bias, np.float32)
    Wo = np.asarray(out_proj_weight, np.float32)
    tau_c = np.maximum(np.asarray(tau, np.float32).reshape(H), TAU_MIN)

    xT = {}
    for b in range(B):
        xT["q", b] = np.ascontiguousarray(query[:, b, :].T).astype(BF16)
        xT["k", b] = np.ascontiguousarray(key[:, b, :].T).astype(BF16)
        xT["v", b] = np.ascontiguousarray(value[:, b, :].T).astype(BF16)

    selq_host = np.zeros((2, 128), np.float32)
    selq_host[0, 0:64] = 1.0
    selq_host[1, 64:128] = 1.0
    in_maps = []
    for c in range(NCORES):
        b = c // 4
        h0 = HPC * (c % 4)
        rows = slice(h0 * HD, (h0 + HPC) * HD)
        rows_k = slice(E + h0 * HD, E + (h0 + HPC) * HD)
        rows_v = slice(2 * E + h0 * HD, 2 * E + (h0 + HPC) * HD)
        # tau^2 per (head-in-pair, pair): sqrt(ss * tau^2) = tau * ||k||,
        # whose reciprocal is the exp scale 1/(tau*||k||).
        tau2i = np.zeros((2, 2), np.float32)
        for mc in range(NPAIR):
            tau2i[0, mc] = tau_c[h0 + 2 * mc] ** 2
            tau2i[1, mc] = tau_c[h0 + 2 * mc + 1] ** 2
        in_maps.append({
            "xq_t": xT["q", b],
            "xk_t": xT["k", b],
            "xv_t": xT["v", b],
            "wq_t": np.ascontiguousarray(W[rows, :].T).astype(BF16),
            "wk_t": np.ascontiguousarray(W[rows_k, :].T).astype(BF16),
            "wv_t": np.ascontiguousarray(W[rows_v, :].T).astype(BF16),
            "b_q": bias[rows].reshape(1, 256).astype(BF16),
            "b_k": bias[rows_k].reshape(1, 256).astype(BF16),
            "b_v": bias[rows_v].reshape(1, 256).astype(BF16),
            "wo_t": np.ascontiguousarray(Wo[:, rows].T).astype(BF16),
            "selq": selq_host,
            "tau2i": tau2i,
        })
    return in_maps


def assemble_out(results, out_proj_bias):
    bo = np.asarray(out_proj_bias, np.float32)
    out = np.zeros((S, B, E), np.float32)
    for c in range(NCORES):
        out[:, c // 4, :] += results[c]["out_p"].astype(np.float32)
    out += bo[None, None, :]
    return out


def kernel(query, key, value, in_proj_weight, in_proj_bias,
           out_proj_weight, out_proj_bias, tau):
    nc = _get_program()
    in_maps = make_in_maps(query, key, value, in_proj_weight, in_proj_bias,
                           out_proj_weight, out_proj_bias, tau)
    res = run_bass_kernel_spmd(nc, in_maps, core_ids=list(range(NCORES)))
    return assemble_out(res.results, out_proj_bias)


if __name__ == "__main__":
    import reference

    inputs = {k: np.asarray(v) for k, v in reference.setup_inputs().items()}
    out = kernel(**inputs)
    print("out shape", out.shape, out.dtype)


# revision 19
# speedup vs baseline: 1.3302x; 1.3302x over previous
"""Cosine multihead attention on 8 Trainium2 NeuronCores.

Sharding: batch*heads across cores. Core c handles batch b = c // 4 and the
4 heads [4*(c%4), 4*(c%4)+4). Each core computes its heads' q/k/v projections
(tensor-parallel slices of in_proj), full attention for its (B,H) slice, and a
partial out-projection (rank-256 contribution, bf16). The host sums the 4
partials per batch in fp32 and adds out_proj_bias.

Key structure (v2):
- q projected transposed [dims, seq], L2-normalized on-chip (bf16).
- k projected transposed but NOT normalized: 1/(tau*||k||) is folded into the
  softmax exp as the ACT per-partition scale operand (partitions = keys), via
  sqrt(ss*tau^-2) -> bf16 -> DMA-transpose -> DVE reciprocal.
- v projected transposed then moved to natural [keys, head, 65] layout with 4
  big DMA-transposes; column 64 holds ones so PV also accumulates softmax
  denominators (M=65).
- Attention in 1024-query superblocks: per (pair, half, kc) two QK matmuls on
  concurrent 64-row PE tiles (1024-wide moving), exp per head on [128,1024],
  then two PV matmuls (1024-wide moving).
- A subset of exp tiles is computed on the Vector engine with a Schraudolph
  bf16 bit-trick (t = s*C/(tau*||k||) + B; round to int16; bits are bf16) to
  take load off the Scalar engine, which is otherwise the bottleneck.
- Out-projection in 1024-wide units, bf16 partial written to HBM.
"""

import sys

if "/opt/trn_rl_repo" not in sys.path:
    sys.path.insert(0, "/opt/trn_rl_repo")

import numpy as np
import ml_dtypes

import concourse.bass as bass
import concourse.tile as tile
from concourse import bacc, mybir
from concourse.bass_utils import run_bass_kernel_spmd

S, B, E, H = 2048, 2, 1024, 16
HD = E // H            # 64
HPC = 4                # heads per core
NCORES = 8
TAU_MIN = 0.01

BF16 = ml_dtypes.bfloat16
DT_BF = mybir.dt.bfloat16
DT_F32 = mybir.dt.float32
DT_I16 = mybir.dt.int16

KC_E = E // 128        # 8 contraction chunks for projections
MQ = S // 128          # 16 seq chunks of 128
NPAIR = HPC // 2       # 2 head pairs per core
NKC = S // 128         # 16 key chunks in attention

# Schraudolph exp-on-DVE: which kc chunks of each (pair, half) round go to the
# vector engine instead of ACT.  4/16 keeps the extra error ~2.4e-3 while
# cutting ACT exp work by 25%.
SCHR_KC = frozenset((3, 7, 11, 14))
SCHR_C = 128.0 * 1.4426950408889634   # 2^7 * log2(e)
SCHR_B = 127.0 * 128.0 - 6.0          # exponent bias - error-balancing shift


def build_program():
    nc = bacc.Bacc(None)

    xq = nc.dram_tensor("xq_t", [E, S], DT_BF, kind="ExternalInput")
    xk = nc.dram_tensor("xk_t", [E, S], DT_BF, kind="ExternalInput")
    xv = nc.dram_tensor("xv_t", [E, S], DT_BF, kind="ExternalInput")
    wq = nc.dram_tensor("wq_t", [E, 256], DT_BF, kind="ExternalInput")
    wk = nc.dram_tensor("wk_t", [E, 256], DT_BF, kind="ExternalInput")
    wv = nc.dram_tensor("wv_t", [E, 256], DT_BF, kind="ExternalInput")
    bq = nc.dram_tensor("b_q", [1, 256], DT_BF, kind="ExternalInput")
    bk = nc.dram_tensor("b_k", [1, 256], DT_BF, kind="ExternalInput")
    bv = nc.dram_tensor("b_v", [1, 256], DT_BF, kind="ExternalInput")
    wo = nc.dram_tensor("wo_t", [256, E], DT_BF, kind="ExternalInput")
    selq_in = nc.dram_tensor("selq", [2, 128], DT_F32, kind="ExternalInput")
    tau2_in = nc.dram_tensor("tau2i", [2, 2], DT_F32, kind="ExternalInput")
    outp = nc.dram_tensor("out_p", [S, E], DT_BF, kind="ExternalOutput")
    DEBUG = bool(__import__("os").environ.get("KDEBUG"))
    if DEBUG:
        dbg_qt = nc.dram_tensor("dbg_qt", [2, 128, S], DT_BF, kind="ExternalOutput")
        dbg_kt = nc.dram_tensor("dbg_kt", [2, 128, S], DT_BF, kind="ExternalOutput")
        dbg_rk2 = nc.dram_tensor("dbg_rk2", [2, NPAIR, S], DT_BF, kind="ExternalOutput")
        dbg_v = nc.dram_tensor("dbg_v", [128, 16, 4, 65], DT_BF, kind="ExternalOutput")
        dbg_ht = nc.dram_tensor("dbg_ht", [2, 128, S], DT_BF, kind="ExternalOutput")

    with tile.TileContext(nc) as tc:
        with (
            tc.tile_pool(name="consts", bufs=1) as consts,
            tc.tile_pool(name="xin", bufs=1) as xin,
            tc.tile_pool(name="xvp", bufs=1) as xvp,
            tc.tile_pool(name="wts", bufs=1) as wts,
            tc.tile_pool(name="qk", bufs=1) as qkpool,
            tc.tile_pool(name="norm", bufs=1) as normpool,
            tc.tile_pool(name="work", bufs=2) as work,
            tc.tile_pool(name="sqp", bufs=2) as sqp,
            tc.tile_pool(name="expool", bufs=4) as expool,
            tc.tile_pool(name="zwork", bufs=1) as zwork,
            tc.tile_pool(name="outs", bufs=2) as outs,
            tc.tile_pool(name="ps_a", bufs=2, space="PSUM") as ps_a,
            tc.tile_pool(name="ps_b", bufs=2, space="PSUM") as ps_b,
        ):
            # ---- constants -------------------------------------------------
            ones_row = consts.tile([1, 1024], DT_BF, tag="ones_row")
            nc.vector.memset(ones_row, 1.0)
            selq = consts.tile([2, 128], DT_F32, tag="selq")
            nc.sync.dma_start(out=selq, in_=selq_in[:, :])
            tau2_sb = consts.tile([2, 2], DT_F32, tag="tau2")
            nc.sync.dma_start(out=tau2_sb, in_=tau2_in[:, :])
            hsel = consts.tile([128, 2], DT_BF, tag="hsel")
            nc.vector.memset(hsel, 0.0)
            nc.vector.memset(hsel[0:64, 0:1], 1.0)
            nc.vector.memset(hsel[64:128, 1:2], 1.0)
            ones_hi = consts.tile([128, 64], DT_F32, tag="ones_hi")
            nc.vector.memset(ones_hi, 1.0)

            # ---- weights ---------------------------------------------------
            wq_sb = wts.tile([128, KC_E, 256], DT_BF, tag="wq")
            wk_sb = wts.tile([128, KC_E, 256], DT_BF, tag="wk")
            wv_sb = wts.tile([128, KC_E, 256], DT_BF, tag="wv")
            bq_sb = consts.tile([1, 256], DT_BF, tag="bq")
            bk_sb = consts.tile([1, 256], DT_BF, tag="bk")
            bv_sb = consts.tile([1, 256], DT_BF, tag="bv")
            nc.sync.dma_start(out=bq_sb, in_=bq[:, :])
            nc.sync.dma_start(out=bk_sb, in_=bk[:, :])
            nc.sync.dma_start(out=bv_sb, in_=bv[:, :])
            for c in range(KC_E):
                nc.sync.dma_start(out=wq_sb[:, c, :], in_=wq[c * 128:(c + 1) * 128, :])
                nc.gpsimd.dma_start(out=wk_sb[:, c, :], in_=wk[c * 128:(c + 1) * 128, :])
                nc.gpsimd.dma_start(out=wv_sb[:, c, :], in_=wv[c * 128:(c + 1) * 128, :])
            wo_sb = wts.tile([128, 2, E], DT_BF, tag="wo")
            for c in range(2):
                nc.gpsimd.dma_start(out=wo_sb[:, c, :], in_=wo[c * 128:(c + 1) * 128, :])

            # ---- activations: xq first (q-proj starts earliest) ------------
            xq_sb = xin.tile([128, KC_E, S], DT_BF, tag="xq")
            xk_sb = xin.tile([128, KC_E, S], DT_BF, tag="xk")
            for c in range(KC_E):
                nc.sync.dma_start(out=xq_sb[:, c, :], in_=xq[c * 128:(c + 1) * 128, :])
                nc.gpsimd.dma_start(out=xk_sb[:, c, :], in_=xk[c * 128:(c + 1) * 128, :])
            # xv shares its pool slot with v_sb later (xv is dead once the
            # transposed v projection has consumed it).
            xv_sb = xvp.tile([128, KC_E, S], DT_BF, tag="xv", name="xv_sb")
            for c in range(KC_E):
                nc.sync.dma_start(out=xv_sb[:, c, :], in_=xv[c * 128:(c + 1) * 128, :])

            qt = [qkpool.tile([128, S], DT_BF, tag=f"qt{p}", name=f"qt{p}")
                  for p in range(NPAIR)]
            kt = [qkpool.tile([128, S], DT_BF, tag=f"kt{p}", name=f"kt{p}")
                  for p in range(NPAIR)]
            heads_t = [qkpool.tile([128, S], DT_BF, tag=f"ht{p}", name=f"ht{p}")
                       for p in range(NPAIR)]
            vT_sb = qkpool.tile([128, 2, S], DT_BF, tag="vT")

            # per-key reciprocal norms 1/(tau*||k||), as rows (bf16)
            rk2 = normpool.tile([2, NPAIR, S], DT_BF, tag="rk2")

            def proj_mms(pp, w_sb, b_sb, x_sb, mcol, half):
                # matmul out must fit one PSUM bank: split free dim in two,
                # sharing the stationary (one LDWEIGHTS per c chunk).
                for c in range(KC_E):
                    for hh in range(2):
                        nc.tensor.matmul(
                            pp[:, hh * 512:(hh + 1) * 512],
                            lhsT=w_sb[:, c, mcol * 128:(mcol + 1) * 128],
                            rhs=x_sb[:, c,
                                     half * 1024 + hh * 512:
                                     half * 1024 + (hh + 1) * 512],
                            start=(c == 0),
                            stop=False,
                        )
                for hh in range(2):
                    nc.tensor.matmul(
                        pp[:, hh * 512:(hh + 1) * 512],
                        lhsT=b_sb[0:1, mcol * 128:(mcol + 1) * 128],
                        rhs=ones_row[0:1, hh * 512:(hh + 1) * 512],
                        start=False,
                        stop=True,
                    )

            def q_unit(mc, half):
                sl = slice(half * 1024, (half + 1) * 1024)
                pp = ps_a.tile([128, 1024], DT_F32, tag="a", name="pp_q")
                proj_mms(pp, wq_sb, bq_sb, xq_sb, mc, half)
                nc.vector.tensor_copy(out=qt[mc][:, sl], in_=pp)
                sq = sqp.tile([128, 1024], DT_BF, tag="sq", name="sq_q")
                nc.vector.tensor_mul(sq, qt[mc][:, sl], qt[mc][:, sl])
                ss = ps_b.tile([2, 1024], DT_F32, tag="b", name="ss_q")
                for hh in range(2):
                    nc.tensor.matmul(
                        ss[:, hh * 512:(hh + 1) * 512], lhsT=hsel,
                        rhs=sq[:, hh * 512:(hh + 1) * 512],
                        start=True, stop=True)
                st = work.tile([2, 1024], DT_F32, tag="st", name="st_q")
                nc.scalar.activation(st, ss, mybir.ActivationFunctionType.Sqrt)
                rb = ps_b.tile([128, 1024], DT_F32, tag="b", name="rb_q")
                for hh in range(2):
                    nc.tensor.matmul(
                        rb[:, hh * 512:(hh + 1) * 512], lhsT=selq,
                        rhs=st[:, hh * 512:(hh + 1) * 512],
                        start=True, stop=True)
                rq = work.tile([128, 1024], DT_F32, tag="rq", name="rq_q")
                nc.vector.reciprocal_approx_fast(out=rq, in_=rb)
                nc.vector.tensor_mul(qt[mc][:, sl], qt[mc][:, sl], rq)

            def k_unit(mc, half):
                sl = slice(half * 1024, (half + 1) * 1024)
                pp = ps_a.tile([128, 1024], DT_F32, tag="a", name="pp_k")
                proj_mms(pp, wk_sb, bk_sb, xk_sb, mc, half)
                nc.vector.tensor_copy(out=kt[mc][:, sl], in_=pp)
                sq = sqp.tile([128, 1024], DT_BF, tag="sq", name="sq_k")
                nc.vector.tensor_mul(sq, kt[mc][:, sl], kt[mc][:, sl])
                ss = ps_b.tile([2, 1024], DT_F32, tag="b", name="ss_k")
                for hh in range(2):
                    nc.tensor.matmul(
                        ss[:, hh * 512:(hh + 1) * 512], lhsT=hsel,
                        rhs=sq[:, hh * 512:(hh + 1) * 512],
                        start=True, stop=True)
                # st_k = sqrt(ss * tau^2) = tau * ||k||, then reciprocal
                st_k = work.tile([2, 1024], DT_F32, tag="st", name="st_k")
                nc.scalar.activation(
                    st_k, ss,
                    mybir.ActivationFunctionType.Sqrt,
                    scale=tau2_sb[:, mc:mc + 1],
                )
                rr = work.tile([2, 1024], DT_F32, tag="st", name="rr_k")
                nc.vector.reciprocal_approx_fast(out=rr, in_=st_k)
                nc.vector.tensor_copy(out=rk2[:, mc, sl], in_=rr)

            def v_unit(d, half):
                sl = slice(half * 1024, (half + 1) * 1024)
                pp = ps_a.tile([128, 1024], DT_F32, tag="a", name="pp_v")
                proj_mms(pp, wv_sb, bv_sb, xv_sb, d, half)
                nc.vector.tensor_copy(out=vT_sb[:, d, sl], in_=pp)

            # ---- all projections up front ----------------------------------
            for half in range(2):
                q_unit(0, half)
            for half in range(2):
                k_unit(0, half)
            for half in range(2):
                q_unit(1, half)
            for half in range(2):
                k_unit(1, half)
            for d in range(2):
                for half in range(2):
                    v_unit(d, half)

            # pre-scale kt rows by 1/(tau*||k||): broadcast each head's
            # reciprocal-norm row across its 64 partitions, then one multiply.
            selqb = consts.tile([2, 128], DT_BF, tag="selqb")
            nc.vector.tensor_copy(out=selqb, in_=selq)
            for p in range(NPAIR):
                # PE-broadcast each head's reciprocal-norm row across its 64
                # partitions, then scale kt in place (reading PSUM directly).
                for half in range(2):
                    sl = slice(half * 1024, (half + 1) * 1024)
                    rbk = ps_a.tile([128, 1024], DT_F32, tag="a", name="rbk")
                    for hh in range(2):
                        nc.tensor.matmul(
                            rbk[:, hh * 512:(hh + 1) * 512],
                            lhsT=selqb,
                            rhs=rk2[:, p,
                                    half * 1024 + hh * 512:
                                    half * 1024 + (hh + 1) * 512],
                            start=True, stop=True,
                        )
                    nc.vector.tensor_mul(kt[p][:, sl], kt[p][:, sl], rbk)

            # v into natural [keys, kc, head, 65] layout; col 64 = ones.
            # DMA transpose needs a contiguous destination, so land in v2
            # first and strided-copy into place on the vector engine.
            v_sb = xvp.tile([128, NKC, HPC, HD + 1], DT_BF, tag="xv", name="v_sb")
            nc.gpsimd.memset(v_sb[:, :, :, HD:HD + 1], 1.0)
            for d in range(2):
                v2 = work.tile([128, NKC, 128], DT_BF, tag="rq", name="v2")
                nc.scalar.dma_start_transpose(out=v2, in_=vT_sb[:, d, :])
                for j in range(2):
                    nc.vector.tensor_copy(
                        out=v_sb[:, :, 2 * d + j, 0:HD],
                        in_=v2[:, :, j * 64:(j + 1) * 64],
                    )

            # ---- attention -------------------------------------------------
            # 512-query rounds; sc holds both heads [128 keys, 2*512 q] and is
            # double-buffered so QK(kc+1) overlaps exp(kc); PV(kc-1) is
            # emitted after QK(kc) so the in-order PE never stalls on the
            # current exp.
            def attention_round(p, qb):
                sl_q = slice(qb * 512, (qb + 1) * 512)
                o = ps_b.tile([65, 1024], DT_F32, tag="b", name="o_acc")
                exs = [None] * NKC

                def qk(kc):
                    scn = ps_a.tile([128, 1024], DT_F32, tag="a", name="sc")
                    for j in range(2):
                        rows = slice(j * 64, (j + 1) * 64)
                        nc.tensor.matmul(
                            scn[:, j * 512:(j + 1) * 512],
                            lhsT=kt[p][rows, kc * 128:(kc + 1) * 128],
                            rhs=qt[p][rows, sl_q],
                            start=True, stop=True,
                        )
                    ex = expool.tile([128, 1024], DT_BF, tag="ex", name="ex")
                    if kc in SCHR_KC:
                        tf = work.tile([128, 1024], DT_F32, tag="rq", name="tf")
                        nc.vector.tensor_scalar(
                            out=tf, in0=scn,
                            scalar1=SCHR_C, scalar2=SCHR_B,
                            op0=mybir.AluOpType.mult,
                            op1=mybir.AluOpType.add,
                        )
                        nc.vector.tensor_copy(out=ex.bitcast(DT_I16), in_=tf)
                    else:
                        nc.scalar.activation(
                            ex, scn, mybir.ActivationFunctionType.Exp
                        )
                    exs[kc] = ex

                def pv(kc):
                    for j in range(2):
                        nc.tensor.matmul(
                            o[0:65, j * 512:(j + 1) * 512],
                            lhsT=v_sb[:, kc, 2 * p + j, :],
                            rhs=exs[kc][:, j * 512:(j + 1) * 512],
                            start=(kc == 0), stop=(kc == NKC - 1),
                        )

                for kc in range(NKC):
                    qk(kc)
                    if kc > 0:
                        pv(kc - 1)
                pv(NKC - 1)

                # normalize: z row (partition 64) -> SBUF (scalar engine),
                # PE-broadcast to partitions 0-63, reciprocal, multiply.
                zs = zwork.tile([128, 1024], DT_F32, tag="rz", name="zs")
                nc.scalar.activation(
                    zs[64:65, :], o[64:65, :],
                    mybir.ActivationFunctionType.Copy,
                )
                zb = ps_b.tile([64, 1024], DT_F32, tag="b", name="zb")
                for hh in range(2):
                    nc.tensor.matmul(
                        zb[:, hh * 512:(hh + 1) * 512],
                        lhsT=ones_hi[64:65, 0:64],
                        rhs=zs[64:65, hh * 512:(hh + 1) * 512],
                        start=True, stop=True,
                    )
                zbi = zwork.tile([64, 1024], DT_F32, tag="zb", name="zbi")
                nc.vector.reciprocal_approx_fast(out=zbi, in_=zb)
                for j in range(2):
                    rows = slice(j * 64, (j + 1) * 64)
                    nc.vector.tensor_mul(
                        heads_t[p][rows, sl_q],
                        o[0:64, j * 512:(j + 1) * 512],
                        zbi[:, j * 512:(j + 1) * 512],
                    )

            def outproj_unit(m):
                op = ps_a.tile([128, 1024], DT_F32, tag="a", name="op")
                for c in range(2):
                    for hh in range(2):
                        nc.tensor.matmul(
                            op[:, hh * 512:(hh + 1) * 512],
                            lhsT=heads_t[c][:, m * 128:(m + 1) * 128],
                            rhs=wo_sb[:, c, hh * 512:(hh + 1) * 512],
                            start=(c == 0), stop=(c == 1),
                        )
                ob = outs.tile([128, 1024], DT_BF, tag="ob", name="ob")
                nc.vector.tensor_copy(ob, op)
                nc.sync.dma_start(out=outp[m * 128:(m + 1) * 128, :], in_=ob)

            if DEBUG:
                for p in range(NPAIR):
                    nc.sync.dma_start(out=dbg_qt[p], in_=qt[p][:, :])
                    nc.sync.dma_start(out=dbg_kt[p], in_=kt[p][:, :])
                nc.sync.dma_start(out=dbg_rk2[:, :, :], in_=rk2[:, :, :])
                nc.sync.dma_start(out=dbg_v[:, :, :, :], in_=v_sb[:, :, :, :])
            for qb in range(4):
                attention_round(0, qb)
            for qb in range(4):
                attention_round(1, qb)
                for m in range(4 * qb, 4 * qb + 4):
                    outproj_unit(m)
            if DEBUG:
                for p in range(NPAIR):
                    nc.sync.dma_start(out=dbg_ht[p], in_=heads_t[p][:, :])
    nc.compile()
    return nc


_CACHE = {}


def _get_program():
    if "nc" not in _CACHE:
        _CACHE["nc"] = build_program()
    return _CACHE["nc"]


def make_in_maps(query, key, value, in_proj_weight, in_proj_bias,
                 out_proj_weight, out_proj_bias, tau):
    query = np.asarray(query, np.float32)
    key = np.asarray(key, np.float32)
    value = np.asarray(value, np.float32)
    W = np.asarray(in_proj_weight, np.float32)
    bias = np.asarray(in_proj_bias, np.float32)
    Wo = np.asarray(out_proj_weight, np.float32)
    tau_c = np.maximum(np.asarray(tau, np.float32).reshape(H), TAU_MIN)

    xT = {}
    for b in range(B):
        xT["q", b] = np.ascontiguousarray(query[:, b, :].T).astype(BF16)
        xT["k", b] = np.ascontiguousarray(key[:, b, :].T).astype(BF16)
        xT["v", b] = np.ascontiguousarray(value[:, b, :].T).astype(BF16)

    selq_host = np.zeros((2, 128), np.float32)
    selq_host[0, 0:64] = 1.0
    selq_host[1, 64:128] = 1.0
    in_maps = []
    for c in range(NCORES):
        b = c // 4
        h0 = HPC * (c % 4)
        rows = slice(h0 * HD, (h0 + HPC) * HD)
        rows_k = slice(E + h0 * HD, E + (h0 + HPC) * HD)
        rows_v = slice(2 * E + h0 * HD, 2 * E + (h0 + HPC) * HD)
        # tau^2 per (head-in-pair, pair): sqrt(ss * tau^2) = tau * ||k||,
        # whose reciprocal is the exp scale 1/(tau*||k||).
        tau2i = np.zeros((2, 2), np.float32)
        for mc in range(NPAIR):
            tau2i[0, mc] = tau_c[h0 + 2 * mc] ** 2
            tau2i[1, mc] = tau_c[h0 + 2 * mc + 1] ** 2
        in_maps.append({
            "xq_t": xT["q", b],
            "xk_t": xT["k", b],
            "xv_t": xT["v", b],
            "wq_t": np.ascontiguousarray(W[rows, :].T).astype(BF16),
            "wk_t": np.ascontiguousarray(W[rows_k, :].T).astype(BF16),
            "wv_t": np.ascontiguousarray(W[rows_v, :].T).astype(BF16),
            "b_q": bias[rows].reshape(1, 256).astype(BF16),
            "b_k": bias[rows_k].reshape(1, 256).astype(BF16),
            "b_v": bias[rows_v].reshape(1, 256).astype(BF16),
            "wo_t": np.ascontiguousarray(Wo[:, rows].T).astype(BF16),
            "selq": selq_host,
            "tau2i": tau2i,
        })
    return in_maps


def assemble_out(results, out_proj_bias):
    bo = np.asarray(out_proj_bias, np.float32)
    out = np.zeros((S, B, E), np.float32)
    for c in range(NCORES):
        out[:, c // 4, :] += results[c]["out_p"].astype(np.float32)
    out += bo[None, None, :]
    return out


def kernel(query, key, value, in_proj_weight, in_proj_bias,
           out_proj_weight, out_proj_bias, tau):
    nc = _get_program()
    in_maps = make_in_maps(query, key, value, in_proj_weight, in_proj_bias,
                           out_proj_weight, out_proj_bias, tau)
    res = run_bass_kernel_spmd(nc, in_maps, core_ids=list(range(NCORES)))
    return assemble_out(res.results, out_proj_bias)


if __name__ == "__main__":
    import reference

    inputs = {k: np.asarray(v) for k, v in reference.setup_inputs().items()}
    out = kernel(**inputs)
    print("out shape", out.shape, out.dtype)
